# revision 5
# baseline (speedup 1.0000x reference)
"""Bahdanau-style additive attention (nn_Attentionv2) on 8 Trainium2 NeuronCores.

Reference computation (per batch b):
    b_img[s,a] = img[s,:] @ Wa^T + Wa_b          # [S, A]
    b_hid[t,a] = hid[t,:] @ Ua^T + Ua_b          # [T, A]
    e[t,s,a]   = tanh(b_img[s,a] + b_hid[t,a])
    scores[t,s]= sum_a va[a]*e[t,s,a] (+ va_b)   # va_b dropped: softmax-invariant
    w          = softmax_s(scores)               # mask is all-ones -> no-op
    context    = w @ img                         # [T, F]
Outputs: context [T,B,F], weights [T,B,S,1].

Sharding: data-parallel over B (8 batches -> 8 cores), one batch per core.

Device-side layout (per core, sizes T=128, S=512, F=H=512, A=256):
  - e is produced in [a_chunk(128 part), (t, s)] layout: the broadcast add
    b_img + b_hid[t] is a DVE tensor_scalar add (per-partition scalar), the
    tanh is one big ScalarE activation per group of t's.
  - scoresT [s(4x128 part), t] via N=1 matmuls: lhsT = tanh tile [a,128 s],
    rhs = va [a,1]; accumulated over the two a-chunks in PSUM.
  - softmax without max-subtraction (|scores| <= sum|va| ~ 8, exp is safe in
    fp32): exp on ScalarE, row sums via PE ones-matmul (partition reduce),
    normalization on DVE.
  - context = wT.T @ img directly (wT is already the needed lhsT layout);
    weights output needs [t,s] so wT is PE-transposed before DMA out.
"""

import numpy as np

T, S, F, H, A = 128, 512, 512, 512, 256
B = 8
NCORES = 8
GT = 16  # t's per tanh group

# precision knobs (validated against the fp32 reference in test.py)
import os as _os

E_DT = _os.environ.get("KERNEL_E_DT", "float32")  # tanh tile dtype; bf16 halves
# DVE add cost (4x mode) and halves PE LDWEIGHTS cost (FWL)

_CACHE = {}


def _build():
    """Build the Bass module (one NeuronCore program, SPMD across 8 cores)."""
    import concourse.bacc as bacc
    import concourse.bass as bass
    import concourse.tile as tile
    from concourse import mybir
    from concourse.masks import make_identity

    f32 = mybir.dt.float32
    e_dt = getattr(mybir.dt, E_DT)

    nc = bacc.Bacc("TRN2")

    imgT_d = nc.declare_dram_parameter("imgT", [F, S], f32, isOutput=False)
    img_d = nc.declare_dram_parameter("img", [S, F], f32, isOutput=False)
    hidT_d = nc.declare_dram_parameter("hidT", [H, T], f32, isOutput=False)
    WaT_d = nc.declare_dram_parameter("WaT", [F, A], f32, isOutput=False)
    UaT_d = nc.declare_dram_parameter("UaT", [H, A], f32, isOutput=False)
    va_d = nc.declare_dram_parameter("va2", [128, 2], e_dt, isOutput=False)
    bias_d = nc.declare_dram_parameter("bias2", [128, 2], f32, isOutput=False)
    ctx_d = nc.declare_dram_parameter("ctx", [T, F], f32, isOutput=True)
    wts_d = nc.declare_dram_parameter("wts", [T, S], f32, isOutput=True)

    with tile.TileContext(nc) as tc:
        with (
            tc.tile_pool(name="const", bufs=1) as const,
            tc.tile_pool(name="work", bufs=2) as work,
            tc.tile_pool(name="misc", bufs=1) as misc,
            tc.tile_pool(name="psum_main", bufs=1, space="PSUM") as pmain,
        ):
            WaT_sb = const.tile([128, 4, A], f32)
            nc.sync.dma_start(WaT_sb[:], WaT_d.rearrange("(c p) a -> p c a", p=128))
            UaT_sb = const.tile([128, 4, A], f32)
            nc.sync.dma_start(UaT_sb[:], UaT_d.rearrange("(c p) a -> p c a", p=128))
            imgT_sb = const.tile([128, 4, S], f32)
            nc.sync.dma_start(imgT_sb[:], imgT_d.rearrange("(c p) s -> p c s", p=128))
            hidT_sb = const.tile([128, 4, T], f32)
            nc.sync.dma_start(hidT_sb[:], hidT_d.rearrange("(c p) t -> p c t", p=128))
            img_sb = const.tile([128, 4, F], f32)
            nc.sync.dma_start(img_sb[:], img_d.rearrange("(c p) f -> p c f", p=128))
            va_sb = const.tile([128, 2], e_dt)
            nc.sync.dma_start(va_sb[:], va_d[:, :])
            bias_sb = const.tile([128, 2], f32)
            nc.sync.dma_start(bias_sb[:], bias_d[:, :])
            ident = const.tile([128, 128], f32)
            make_identity(nc, ident[:])
            ones_sb = const.tile([128, 128], f32)
            nc.vector.memset(ones_sb[:], 1.0)

            bimg_sb = misc.tile([128, 2, S], e_dt)  # [a_chunk part, (chunk, s)]
            bhid_sb = misc.tile([128, 2, T], f32)  # [a_chunk part, (chunk, t)]

            # prologue: b_img = Wa@img^T, b_hid = Ua@hid^T (+ combined bias)
            with tc.tile_pool(name="psum_pre", bufs=2, space="PSUM") as pp:
                for c in range(2):
                    t_img = pp.tile([128, S], f32, tag="pimg")
                    for k in range(4):
                        nc.tensor.matmul(
                            t_img[:],
                            WaT_sb[:, k, c * 128 : (c + 1) * 128],
                            imgT_sb[:, k, :],
                            start=(k == 0),
                            stop=(k == 3),
                        )
                    nc.vector.tensor_copy(bimg_sb[:, c, :], t_img[:])
                    t_hid = pp.tile([128, T], f32, tag="phid")
                    for k in range(4):
                        nc.tensor.matmul(
                            t_hid[:],
                            UaT_sb[:, k, c * 128 : (c + 1) * 128],
                            hidT_sb[:, k, :],
                            start=(k == 0),
                            stop=(k == 3),
                        )
                    # bias (Wa_b + Ua_b) folded into b_hid during PSUM->SBUF copy
                    nc.scalar.add(bhid_sb[:, c, :], t_hid[:], bias_sb[:, c : c + 1])

            # scoresT[s, t] accumulates here: [s%128 part, (s_blk, t)] = 1 bank
            scT = pmain.tile([128, 4, T], f32)

            # main loop: groups of GT t's
            for g in range(T // GT):
                arg = work.tile([128, GT, 2, S], e_dt, tag="arg")
                for i in range(GT):
                    t = g * GT + i
                    for c in range(2):
                        # arg[a, s] = b_img[a, s] + b_hid[a, t]
                        nc.vector.tensor_scalar_add(
                            arg[:, i, c, :], bimg_sb[:, c, :], bhid_sb[:, c, t : t + 1]
                        )
                # one big tanh per group (in place)
                nc.scalar.activation(
                    arg[:], arg[:], mybir.ActivationFunctionType.Tanh
                )
                # va-reduce: scoresT[:, sb, t] = sum_a va[a] * e[a, t, sb*128: ...]
                for i in range(GT):
                    t = g * GT + i
                    for sb in range(4):
                        for c in range(2):
                            nc.tensor.matmul(
                                scT[:, sb, t : t + 1],
                                arg[:, i, c, sb * 128 : (sb + 1) * 128],
                                va_sb[:, c : c + 1],
                                start=(c == 0),
                                stop=(c == 1),
                            )

            # softmax (no max subtraction: |scores| <= sum|va| << fp32 range)
            expT_sb = misc.tile([128, 4, T], f32)
            nc.scalar.activation(
                expT_sb[:], scT[:], mybir.ActivationFunctionType.Exp
            )
            with tc.tile_pool(name="psum_post", bufs=2, space="PSUM") as pe:
                sums_ps = pe.tile([128, T], f32, tag="sums")
                for c in range(4):
                    nc.tensor.matmul(
                        sums_ps[:],
                        ones_sb[:],
                        expT_sb[:, c, :],
                        start=(c == 0),
                        stop=(c == 3),
                    )
                recip_sb = misc.tile([128, T], f32)
                nc.vector.reciprocal(recip_sb[:], sums_ps[:])
                wT_sb = misc.tile([128, 4, T], f32)
                for c in range(4):
                    nc.vector.tensor_mul(
                        wT_sb[:, c, :], expT_sb[:, c, :], recip_sb[:]
                    )

                # context[t, f] = sum_s w[t, s] img[s, f]; lhsT = wT directly
                ctx_ps = pe.tile([128, F], f32, tag="ctx")
                for c in range(4):
                    nc.tensor.matmul(
                        ctx_ps[:],
                        wT_sb[:, c, :],
                        img_sb[:, c, :],
                        start=(c == 0),
                        stop=(c == 3),
                    )
                ctx_sb = misc.tile([128, F], f32)
                nc.vector.tensor_copy(ctx_sb[:], ctx_ps[:])
                nc.sync.dma_start(ctx_d[:, :], ctx_sb[:])

                # weights output needs [t, s]: PE-transpose each wT block
                w_sb = misc.tile([128, 4, 128], f32)
                for c in range(4):
                    wtp = pe.tile([128, 128], f32, tag="wtp")
                    nc.tensor.transpose(wtp[:], wT_sb[:, c, :], ident[:])
                    nc.vector.tensor_copy(w_sb[:, c, :], wtp[:])
                nc.sync.dma_start(
                    wts_d[:, :], w_sb[:].rearrange("p c s -> p (c s)")
                )

    nc.compile()
    return nc


def _get_nc():
    if "nc" not in _CACHE:
        _CACHE["nc"] = _build()
    return _CACHE["nc"]


def make_in_maps(last_hiddens, image_features, Wa_w, Wa_b, Ua_w, Ua_b, va_w):
    """Host-side sharding + pre-transposes. One map per core (= per batch)."""
    f = np.float32
    if E_DT == "float32":
        e_np = np.float32
    else:
        import ml_dtypes

        e_np = getattr(ml_dtypes, E_DT)
    WaT = np.ascontiguousarray(Wa_w.T, dtype=f)  # [F, A]
    UaT = np.ascontiguousarray(Ua_w.T, dtype=f)  # [H, A]
    va2 = np.ascontiguousarray(va_w[0].reshape(2, 128).T).astype(e_np)  # [128, 2]
    bias2 = np.ascontiguousarray(
        (Wa_b + Ua_b).reshape(2, 128).T, dtype=f
    )  # [128, 2]
    in_maps = []
    for b in range(NCORES):
        img_b = np.ascontiguousarray(image_features[:, b, :], dtype=f)  # [S, F]
        in_maps.append(
            {
                "imgT": np.ascontiguousarray(img_b.T),  # [F, S]
                "img": img_b,
                "hidT": np.ascontiguousarray(last_hiddens[:, b, :].T, dtype=f),
                "WaT": WaT,
                "UaT": UaT,
                "va2": va2,
                "bias2": bias2,
            }
        )
    return in_maps


def _assemble(results):
    context = np.empty((T, B, F), np.float32)
    weights = np.empty((T, B, S, 1), np.float32)
    for b in range(B):
        context[:, b, :] = results[b]["ctx"]
        weights[:, b, :, 0] = results[b]["wts"]
    return context, weights


def run(inputs, trace=False, **spmd_kwargs):
    """Compile (cached) + run on cores 0-7. Returns (outputs, BassKernelResults)."""
    from concourse.bass_utils import run_bass_kernel_spmd

    in_maps = make_in_maps(
        np.asarray(inputs["last_hiddens"]),
        np.asarray(inputs["image_features"]),
        np.asarray(inputs["Wa_w"]),
        np.asarray(inputs["Wa_b"]),
        np.asarray(inputs["Ua_w"]),
        np.asarray(inputs["Ua_b"]),
        np.asarray(inputs["va_w"]),
    )
    nc = _get_nc()
    res = run_bass_kernel_spmd(
        nc, in_maps, core_ids=list(range(NCORES)), trace=trace, **spmd_kwargs
    )
    return _assemble(res.results), res


def kernel(**inputs):
    outputs, _ = run(inputs)
    return outputs


# revision 16
# speedup vs baseline: 1.0509x; 1.0509x over previous
"""Bahdanau-style additive attention (nn_Attentionv2) on 8 Trainium2 NeuronCores.

Reference computation (per batch b):
    b_img[s,a] = img[s,:] @ Wa^T + Wa_b          # [S, A]
    b_hid[t,a] = hid[t,:] @ Ua^T + Ua_b          # [T, A]
    e[t,s,a]   = tanh(b_img[s,a] + b_hid[t,a])
    scores[t,s]= sum_a va[a]*e[t,s,a] (+ va_b)   # va_b dropped: softmax-invariant
    w          = softmax_s(scores)               # mask is all-ones -> no-op
    context    = w @ img                         # [T, F]
Outputs: context [T,B,F], weights [T,B,S,1].

Sharding: data-parallel over B (8 batches -> 8 cores), one batch per core.

Per-core design (T=128, S=512, F=H=512, A=256). The kernel is ScalarE-bound:
16.8M tanh evaluations at 128 lanes / 1.2 GHz ~= 110us is the floor, so the
structure keeps ACT saturated and hides everything else under it:
  - e in [a_chunk(128 part), (t, s)] layout: broadcast add b_img + b_hid[t]
    is a DVE tensor_scalar (per-partition scalar, bf16 4x mode), tanh is one
    big ACT instruction per group of t's (in place).
  - group sizes ramp up 4,4,8,16,... so ACT starts early, and ramp down at
    the end so the post-tanh tail is short.
  - scoresT [s(4x128 part), t] via N=1 matmuls: lhsT = e tile [a,128 s]
    (bf16 -> single pass + fast weight load), rhs = va [a,1], accumulated
    over the two a-chunks in PSUM.
  - softmax without max-subtraction (|scores| <= sum|va| ~ 8; exp is safe in
    fp32): exp on ACT, row sums via PE ones-matmul (partition reduce),
    normalization on DVE. Done in two t-halves so the first half overlaps
    the second half of the main loop.
  - context = wT.T @ img directly (wT is already the needed lhsT layout);
    the weights output needs [t,s] so wT is PE-transposed before DMA out.
"""

import numpy as np

T, S, F, H, A = 128, 512, 512, 512, 256
B = 8
NCORES = 8

# group sizes for the tanh pipeline (sum = T)
GROUPS = [4, 4, 8, 16, 16, 16, 16, 16, 16, 8, 4, 4]
assert sum(GROUPS) == T
GT_MAX = max(GROUPS)

# precision knobs (validated against the fp32 reference in test.py)
import os as _os

E_DT = _os.environ.get("KERNEL_E_DT", "float16")  # tanh tile dtype
PRE_F32R = _os.environ.get("KERNEL_PRE_F32R", "0") == "1"  # single-pass
# fp32 (tf32-ish) for the prologue Wa/Ua matmuls

_CACHE = {}


def _build():
    """Build the Bass module (one NeuronCore program, SPMD across 8 cores)."""
    import concourse.bacc as bacc
    import concourse.bass as bass
    import concourse.tile as tile
    from concourse import mybir
    from concourse.masks import make_identity

    f32 = mybir.dt.float32
    e_dt = getattr(mybir.dt, E_DT)

    def pre(ap):
        return ap.bitcast(mybir.dt.float32r) if PRE_F32R else ap

    nc = bacc.Bacc("TRN2")

    imgT_d = nc.declare_dram_parameter("imgT", [F, S], f32, isOutput=False)
    img_d = nc.declare_dram_parameter("img", [S, F], f32, isOutput=False)
    hidT_d = nc.declare_dram_parameter("hidT", [H, T], f32, isOutput=False)
    WaT_d = nc.declare_dram_parameter("WaT", [F, A], f32, isOutput=False)
    UaT_d = nc.declare_dram_parameter("UaT", [H, A], f32, isOutput=False)
    va_d = nc.declare_dram_parameter("va2", [128, 2], e_dt, isOutput=False)
    bias_d = nc.declare_dram_parameter("bias2", [128, 2], f32, isOutput=False)
    ctx_d = nc.declare_dram_parameter("ctx", [T, F], f32, isOutput=True)
    wts_d = nc.declare_dram_parameter("wts", [T, S], f32, isOutput=True)

    with tile.TileContext(nc) as tc:
        with (
            tc.tile_pool(name="const", bufs=1) as const,
            tc.tile_pool(name="work", bufs=3) as work,
            tc.tile_pool(name="misc", bufs=1) as misc,
        ):
            # loads needed for b_hid (small, first) and b_img (the ACT-start
            # critical path), then the context-matmul rhs (not urgent)
            hidT_sb = const.tile([128, 4, T], f32)
            nc.sync.dma_start(hidT_sb[:], hidT_d.rearrange("(c p) t -> p c t", p=128))
            UaT_sb = const.tile([128, 4, A], f32)
            nc.sync.dma_start(UaT_sb[:], UaT_d.rearrange("(c p) a -> p c a", p=128))
            WaT_sb = const.tile([128, 4, A], f32)
            nc.sync.dma_start(WaT_sb[:], WaT_d.rearrange("(c p) a -> p c a", p=128))
            imgT_sb = const.tile([128, 4, S], f32)
            nc.sync.dma_start(imgT_sb[:], imgT_d.rearrange("(c p) s -> p c s", p=128))
            va_sb = const.tile([128, 2], e_dt)
            nc.sync.dma_start(va_sb[:], va_d[:, :])
            bias_sb = const.tile([128, 2], f32)
            nc.sync.dma_start(bias_sb[:], bias_d[:, :])
            img_sb = const.tile([128, 4, F], f32)
            nc.sync.dma_start(img_sb[:], img_d.rearrange("(c p) f -> p c f", p=128))
            ident = const.tile([128, 128], f32)
            make_identity(nc, ident[:])
            ones_sb = const.tile([128, 128], f32)
            nc.vector.memset(ones_sb[:], 1.0)

            bimg_sb = misc.tile([128, 2, S], e_dt)  # [a_chunk part, (chunk, s)]
            bhid_sb = misc.tile([128, 2, T], f32)  # [a_chunk part, (chunk, t)]

            # prologue: b_img = Wa@img^T, b_hid = Ua@hid^T (+ combined bias)
            with tc.tile_pool(name="psum_pre", bufs=2, space="PSUM") as pp:
                for c in range(2):
                    t_hid = pp.tile([128, T], f32, tag="phid")
                    for k in range(4):
                        nc.tensor.matmul(
                            t_hid[:],
                            pre(UaT_sb[:, k, c * 128 : (c + 1) * 128]),
                            pre(hidT_sb[:, k, :]),
                            start=(k == 0),
                            stop=(k == 3),
                        )
                    # bias (Wa_b + Ua_b) folded into b_hid in the PSUM->SBUF copy
                    nc.scalar.add(bhid_sb[:, c, :], t_hid[:], bias_sb[:, c : c + 1])
                    t_img = pp.tile([128, S], f32, tag="pimg")
                    for k in range(4):
                        nc.tensor.matmul(
                            t_img[:],
                            pre(WaT_sb[:, k, c * 128 : (c + 1) * 128]),
                            pre(imgT_sb[:, k, :]),
                            start=(k == 0),
                            stop=(k == 3),
                        )
                    nc.vector.tensor_copy(bimg_sb[:, c, :], t_img[:])

            pmain_cm = tc.tile_pool(name="psum_main", bufs=1, space="PSUM")
            pmain = pmain_cm.__enter__()
            # scoresT[s, t] accumulates here: [s%128 part, (s_blk, t)] = 1 bank
            scT = pmain.tile([128, 4, T], f32)

            # epilogue state (written per t-half)
            expT_sb = misc.tile([128, 4, T], f32)
            recip_sb = misc.tile([128, T], f32)
            wT_sb = misc.tile([128, 4, T], f32)

            def epilogue_half(h):
                tsl = slice(h * 64, h * 64 + 64)
                # softmax (no max subtraction): exp reads scoresT from PSUM
                nc.scalar.activation(
                    expT_sb[:, :, tsl], scT[:, :, tsl],
                    mybir.ActivationFunctionType.Exp,
                )
                sums_ps = pmain.tile([128, 64], f32, tag=f"sums{h}")
                for c in range(4):
                    nc.tensor.matmul(
                        sums_ps[:],
                        ones_sb[:],
                        expT_sb[:, c, tsl],
                        start=(c == 0),
                        stop=(c == 3),
                    )
                nc.vector.reciprocal(recip_sb[:, tsl], sums_ps[:])
                for c in range(4):
                    nc.vector.tensor_mul(
                        wT_sb[:, c, tsl], expT_sb[:, c, tsl], recip_sb[:, tsl]
                    )
                # context[t, f] = sum_s w[t, s] img[s, f]; lhsT = wT directly.
                # PSUM outputs sit at partition 0; the DMA places rows t.
                ctx_ps = pmain.tile([64, F], f32, tag=f"ctx{h}")
                for c in range(4):
                    nc.tensor.matmul(
                        ctx_ps[:],
                        wT_sb[:, c, tsl],
                        img_sb[:, c, :],
                        start=(c == 0),
                        stop=(c == 3),
                    )
                ctx_sb = misc.tile([64, F], f32, tag=f"ctxsb{h}")
                nc.vector.tensor_copy(ctx_sb[:], ctx_ps[:])
                nc.sync.dma_start(ctx_d[tsl, :], ctx_sb[:])
                # weights output needs [t, s]: PE-transpose each wT block
                wtp_ps = pmain.tile([64, 4, 128], f32, tag=f"wtp{h}")
                w_sb = misc.tile([64, 4, 128], f32, tag=f"wsb{h}")
                for c in range(4):
                    nc.tensor.transpose(
                        wtp_ps[:, c, :], wT_sb[:, c, tsl], ident[:]
                    )
                    nc.vector.tensor_copy(w_sb[:, c, :], wtp_ps[:, c, :])
                nc.sync.dma_start(
                    wts_d[tsl, :], w_sb[:].rearrange("p c s -> p (c s)")
                )

            # main loop over t-groups
            t0 = 0
            for gi, gt in enumerate(GROUPS):
                arg = work.tile([128, GT_MAX, 2, S], e_dt, tag="arg")
                for i in range(gt):
                    t = t0 + i
                    for c in range(2):
                        # arg[a, s] = b_img[a, s] + b_hid[a, t]
                        nc.vector.tensor_scalar_add(
                            arg[:, i, c, :], bimg_sb[:, c, :], bhid_sb[:, c, t : t + 1]
                        )
                # one tanh instruction per group (in place)
                nc.scalar.activation(
                    arg[:, 0:gt, :, :], arg[:, 0:gt, :, :],
                    mybir.ActivationFunctionType.Tanh,
                )
                # va-reduce: scoresT[:, sb, t] = sum_a va[a] * e[a, t, sb]
                for i in range(gt):
                    t = t0 + i
                    for sb in range(4):
                        for c in range(2):
                            nc.tensor.matmul(
                                scT[:, sb, t : t + 1],
                                arg[:, i, c, sb * 128 : (sb + 1) * 128],
                                va_sb[:, c : c + 1],
                                start=(c == 0),
                                stop=(c == 1),
                            )
                t0 += gt
                if t0 == 64:
                    epilogue_half(0)
            epilogue_half(1)
            pmain_cm.__exit__(None, None, None)

    nc.compile()
    return nc


def _get_nc():
    if "nc" not in _CACHE:
        _CACHE["nc"] = _build()
    return _CACHE["nc"]


def make_in_maps(last_hiddens, image_features, Wa_w, Wa_b, Ua_w, Ua_b, va_w):
    """Host-side sharding + pre-transposes. One map per core (= per batch)."""
    f = np.float32
    if E_DT == "float32":
        e_np = np.float32
    elif E_DT == "float16":
        e_np = np.float16
    else:
        import ml_dtypes

        e_np = getattr(ml_dtypes, E_DT)
    WaT = np.ascontiguousarray(Wa_w.T, dtype=f)  # [F, A]
    UaT = np.ascontiguousarray(Ua_w.T, dtype=f)  # [H, A]
    va2 = np.ascontiguousarray(va_w[0].reshape(2, 128).T).astype(e_np)  # [128, 2]
    bias2 = np.ascontiguousarray(
        (Wa_b + Ua_b).reshape(2, 128).T, dtype=f
    )  # [128, 2]
    in_maps = []
    for b in range(NCORES):
        img_b = np.ascontiguousarray(image_features[:, b, :], dtype=f)  # [S, F]
        in_maps.append(
            {
                "imgT": np.ascontiguousarray(img_b.T),  # [F, S]
                "img": img_b,
                "hidT": np.ascontiguousarray(last_hiddens[:, b, :].T, dtype=f),
                "WaT": WaT,
                "UaT": UaT,
                "va2": va2,
                "bias2": bias2,
            }
        )
    return in_maps


def _assemble(results):
    context = np.empty((T, B, F), np.float32)
    weights = np.empty((T, B, S, 1), np.float32)
    for b in range(B):
        context[:, b, :] = results[b]["ctx"]
        weights[:, b, :, 0] = results[b]["wts"]
    return context, weights


def run(inputs, trace=False, **spmd_kwargs):
    """Compile (cached) + run on cores 0-7. Returns (outputs, BassKernelResults)."""
    from concourse.bass_utils import run_bass_kernel_spmd

    in_maps = make_in_maps(
        np.asarray(inputs["last_hiddens"]),
        np.asarray(inputs["image_features"]),
        np.asarray(inputs["Wa_w"]),
        np.asarray(inputs["Wa_b"]),
        np.asarray(inputs["Ua_w"]),
        np.asarray(inputs["Ua_b"]),
        np.asarray(inputs["va_w"]),
    )
    nc = _get_nc()
    res = run_bass_kernel_spmd(
        nc, in_maps, core_ids=list(range(NCORES)), trace=trace, **spmd_kwargs
    )
    return _assemble(res.results), res


def kernel(**inputs):
    outputs, _ = run(inputs)
    return outputs


# revision 17
# speedup vs baseline: 1.1381x; 1.0830x over previous
"""Bahdanau-style additive attention (nn_Attentionv2) on 8 Trainium2 NeuronCores.

Reference computation (per batch b):
    b_img[s,a] = img[s,:] @ Wa^T + Wa_b          # [S, A]
    b_hid[t,a] = hid[t,:] @ Ua^T + Ua_b          # [T, A]
    e[t,s,a]   = tanh(b_img[s,a] + b_hid[t,a])
    scores[t,s]= sum_a va[a]*e[t,s,a] (+ va_b)   # va_b dropped: softmax-invariant
    w          = softmax_s(scores)               # mask is all-ones -> no-op
    context    = w @ img                         # [T, F]
Outputs: context [T,B,F], weights [T,B,S,1].

Sharding: data-parallel over B (8 batches -> 8 cores), one batch per core.

Per-core design (T=128, S=512, F=H=512, A=256). The kernel is ScalarE-bound:
16.8M tanh evaluations at 128 lanes / 1.2 GHz ~= 110us is the floor, so the
structure keeps ACT saturated from ~10us on and hides everything else:
  - fp16 throughout the attention pipeline (fp16 keeps ~11 mantissa bits on
    values that all live in [-10, 10]; measured end-to-end rel err ~3e-4).
    fp16/bf16 matmuls are single-pass on the PE (fp32 lowers to LOW/HIGH
    double passes) and get fast weight loads.
  - a dummy activation at t=0 pulls the ~2.7us ACT table load off the
    critical path.
  - e in [a_chunk(128 part), (t, s)] layout: broadcast add b_img + b_hid[t]
    is a DVE tensor_scalar (per-partition scalar), tanh is one big ACT
    instruction per group of t's (in place). Group sizes ramp 2,2,4,8,16...
    so ACT starts early, and ramp down at the end to shorten the tail.
  - scoresT [s(4x128 part), t] via N=1 matmuls: lhsT = e tile [a,128 s],
    rhs = va [a,1], accumulated over the two a-chunks in PSUM.
  - softmax without max-subtraction (|scores| <= sum|va| ~ 8; exp is safe in
    fp32): exp on ACT, row sums via PE ones-matmul (partition reduce),
    normalization on DVE. Done in two t-halves so the first half overlaps
    the second half of the main loop.
  - context = wT.T @ img directly (wT is already the needed lhsT layout);
    the weights output needs [t,s] so wT is PE-transposed before DMA out.
"""

import numpy as np

T, S, F, H, A = 128, 512, 512, 512, 256
B = 8
NCORES = 8

# group sizes for the tanh pipeline (sum = T)
GROUPS = [2, 2, 4, 8] + [16] * 6 + [8, 4, 2, 2]
assert sum(GROUPS) == T
GT_MAX = max(GROUPS)

_CACHE = {}


def _build():
    """Build the Bass module (one NeuronCore program, SPMD across 8 cores)."""
    import concourse.bacc as bacc
    import concourse.bass as bass
    import concourse.tile as tile
    from concourse import mybir
    from concourse.masks import make_identity

    f32 = mybir.dt.float32
    f16 = mybir.dt.float16

    nc = bacc.Bacc("TRN2")

    # fp16 inputs (host-converted): prologue + context operands
    imgT_d = nc.declare_dram_parameter("imgT", [F, S], f16, isOutput=False)
    img_d = nc.declare_dram_parameter("img", [S, F], f16, isOutput=False)
    hidT_d = nc.declare_dram_parameter("hidT", [H, T], f16, isOutput=False)
    WaT_d = nc.declare_dram_parameter("WaT", [F, A], f16, isOutput=False)
    UaT_d = nc.declare_dram_parameter("UaT", [H, A], f16, isOutput=False)
    va_d = nc.declare_dram_parameter("va2", [128, 2], f16, isOutput=False)
    bias_d = nc.declare_dram_parameter("bias2", [128, 2], f32, isOutput=False)
    ctx_d = nc.declare_dram_parameter("ctx", [T, F], f32, isOutput=True)
    wts_d = nc.declare_dram_parameter("wts", [T, S], f32, isOutput=True)

    with tile.TileContext(nc) as tc:
        with (
            tc.tile_pool(name="const", bufs=1) as const,
            tc.tile_pool(name="work", bufs=3) as work,
            tc.tile_pool(name="misc", bufs=1) as misc,
        ):
            # dummy activation: pull the ACT table load to t=0
            dummy = const.tile([128, 1], f32)
            nc.vector.memset(dummy[:], 0.0)
            nc.scalar.activation(dummy[:], dummy[:], mybir.ActivationFunctionType.Tanh)

            # loads needed for b_hid (small, first) and b_img (the ACT-start
            # critical path), then the context-matmul rhs (not urgent)
            hidT_sb = const.tile([128, 4, T], f16)
            nc.sync.dma_start(hidT_sb[:], hidT_d.rearrange("(c p) t -> p c t", p=128))
            UaT_sb = const.tile([128, 4, A], f16)
            nc.sync.dma_start(UaT_sb[:], UaT_d.rearrange("(c p) a -> p c a", p=128))
            WaT_sb = const.tile([128, 4, A], f16)
            nc.sync.dma_start(WaT_sb[:], WaT_d.rearrange("(c p) a -> p c a", p=128))
            imgT_sb = const.tile([128, 4, S], f16)
            nc.sync.dma_start(imgT_sb[:], imgT_d.rearrange("(c p) s -> p c s", p=128))
            va_sb = const.tile([128, 2], f16)
            nc.sync.dma_start(va_sb[:], va_d[:, :])
            bias_sb = const.tile([128, 2], f32)
            nc.sync.dma_start(bias_sb[:], bias_d[:, :])
            img_sb = const.tile([128, 4, F], f16)
            nc.sync.dma_start(img_sb[:], img_d.rearrange("(c p) f -> p c f", p=128))
            ident = const.tile([128, 128], f16)
            make_identity(nc, ident[:])
            ones_sb = const.tile([128, 128], f16)
            nc.vector.memset(ones_sb[:], 1.0)

            bimg_sb = misc.tile([128, 2, S], f16)  # [a_chunk part, (chunk, s)]
            bhid_sb = misc.tile([128, 2, T], f32)  # [a_chunk part, (chunk, t)]

            # prologue: b_img = Wa@img^T, b_hid = Ua@hid^T (+ combined bias)
            with tc.tile_pool(name="psum_pre", bufs=2, space="PSUM") as pp:
                for c in range(2):
                    t_hid = pp.tile([128, T], f32, tag="phid")
                    for k in range(4):
                        nc.tensor.matmul(
                            t_hid[:],
                            UaT_sb[:, k, c * 128 : (c + 1) * 128],
                            hidT_sb[:, k, :],
                            start=(k == 0),
                            stop=(k == 3),
                        )
                    # bias (Wa_b + Ua_b) folded into b_hid in the PSUM->SBUF copy
                    nc.vector.tensor_scalar_add(
                        bhid_sb[:, c, :], t_hid[:], bias_sb[:, c : c + 1]
                    )
                    t_img = pp.tile([128, S], f32, tag="pimg")
                    for k in range(4):
                        nc.tensor.matmul(
                            t_img[:],
                            WaT_sb[:, k, c * 128 : (c + 1) * 128],
                            imgT_sb[:, k, :],
                            start=(k == 0),
                            stop=(k == 3),
                        )
                    nc.vector.tensor_copy(bimg_sb[:, c, :], t_img[:])

            pmain_cm = tc.tile_pool(name="psum_main", bufs=1, space="PSUM")
            pmain = pmain_cm.__enter__()
            # scoresT[s, t] accumulates here: [s%128 part, (s_blk, t)] = 1 bank
            scT = pmain.tile([128, 4, T], f32)

            # epilogue state (written per t-half)
            expT_sb = misc.tile([128, 4, T], f16)
            recip_sb = misc.tile([128, T], f32)
            wT_sb = misc.tile([128, 4, T], f16)

            def epilogue_half(h):
                tsl = slice(h * 64, h * 64 + 64)
                # softmax (no max subtraction): exp reads scoresT from PSUM
                nc.scalar.activation(
                    expT_sb[:, :, tsl], scT[:, :, tsl],
                    mybir.ActivationFunctionType.Exp,
                )
                sums_ps = pmain.tile([128, 64], f32, tag=f"sums{h}")
                for c in range(4):
                    nc.tensor.matmul(
                        sums_ps[:],
                        ones_sb[:],
                        expT_sb[:, c, tsl],
                        start=(c == 0),
                        stop=(c == 3),
                    )
                nc.vector.reciprocal(recip_sb[:, tsl], sums_ps[:])
                for c in range(4):
                    nc.vector.tensor_mul(
                        wT_sb[:, c, tsl], expT_sb[:, c, tsl], recip_sb[:, tsl]
                    )
                # context[t, f] = sum_s w[t, s] img[s, f]; lhsT = wT directly.
                # PSUM outputs sit at partition 0; the DMA places rows t.
                ctx_ps = pmain.tile([64, F], f32, tag=f"ctx{h}")
                for c in range(4):
                    nc.tensor.matmul(
                        ctx_ps[:],
                        wT_sb[:, c, tsl],
                        img_sb[:, c, :],
                        start=(c == 0),
                        stop=(c == 3),
                    )
                ctx_sb = misc.tile([64, F], f32, tag=f"ctxsb{h}")
                nc.vector.tensor_copy(ctx_sb[:], ctx_ps[:])
                nc.sync.dma_start(ctx_d[tsl, :], ctx_sb[:])
                # weights output needs [t, s]: PE-transpose each wT block
                wtp_ps = pmain.tile([64, 4, 128], f16, tag=f"wtp{h}")
                w_sb = misc.tile([64, 4, 128], f32, tag=f"wsb{h}")
                for c in range(4):
                    nc.tensor.transpose(
                        wtp_ps[:, c, :], wT_sb[:, c, tsl], ident[:]
                    )
                    nc.vector.tensor_copy(w_sb[:, c, :], wtp_ps[:, c, :])
                nc.sync.dma_start(
                    wts_d[tsl, :], w_sb[:].rearrange("p c s -> p (c s)")
                )

            # main loop over t-groups
            t0 = 0
            for gi, gt in enumerate(GROUPS):
                arg = work.tile([128, GT_MAX, 2, S], f16, tag="arg")
                for i in range(gt):
                    t = t0 + i
                    for c in range(2):
                        # arg[a, s] = b_img[a, s] + b_hid[a, t]
                        nc.vector.tensor_scalar_add(
                            arg[:, i, c, :], bimg_sb[:, c, :], bhid_sb[:, c, t : t + 1]
                        )
                # one tanh instruction per group (in place)
                nc.scalar.activation(
                    arg[:, 0:gt, :, :], arg[:, 0:gt, :, :],
                    mybir.ActivationFunctionType.Tanh,
                )
                # va-reduce: scoresT[:, sb, t] = sum_a va[a] * e[a, t, sb]
                for i in range(gt):
                    t = t0 + i
                    for sb in range(4):
                        for c in range(2):
                            nc.tensor.matmul(
                                scT[:, sb, t : t + 1],
                                arg[:, i, c, sb * 128 : (sb + 1) * 128],
                                va_sb[:, c : c + 1],
                                start=(c == 0),
                                stop=(c == 1),
                            )
                t0 += gt
                if t0 == 64:
                    epilogue_half(0)
            epilogue_half(1)
            pmain_cm.__exit__(None, None, None)

    nc.compile()
    return nc


def _get_nc():
    if "nc" not in _CACHE:
        _CACHE["nc"] = _build()
    return _CACHE["nc"]


def make_in_maps(last_hiddens, image_features, Wa_w, Wa_b, Ua_w, Ua_b, va_w):
    """Host-side sharding + pre-transposes. One map per core (= per batch)."""
    f = np.float32
    h = np.float16
    WaT = np.ascontiguousarray(Wa_w.T).astype(h)  # [F, A]
    UaT = np.ascontiguousarray(Ua_w.T).astype(h)  # [H, A]
    va2 = np.ascontiguousarray(va_w[0].reshape(2, 128).T).astype(h)  # [128, 2]
    bias2 = np.ascontiguousarray(
        (Wa_b + Ua_b).reshape(2, 128).T, dtype=f
    )  # [128, 2]
    in_maps = []
    for b in range(NCORES):
        img_b = np.ascontiguousarray(image_features[:, b, :])  # [S, F]
        in_maps.append(
            {
                "imgT": np.ascontiguousarray(img_b.T).astype(h),  # [F, S]
                "img": img_b.astype(h),
                "hidT": np.ascontiguousarray(last_hiddens[:, b, :].T).astype(h),
                "WaT": WaT,
                "UaT": UaT,
                "va2": va2,
                "bias2": bias2,
            }
        )
    return in_maps


def _assemble(results):
    context = np.empty((T, B, F), np.float32)
    weights = np.empty((T, B, S, 1), np.float32)
    for b in range(B):
        context[:, b, :] = results[b]["ctx"]
        weights[:, b, :, 0] = results[b]["wts"]
    return context, weights


def run(inputs, trace=False, **spmd_kwargs):
    """Compile (cached) + run on cores 0-7. Returns (outputs, BassKernelResults)."""
    from concourse.bass_utils import run_bass_kernel_spmd

    in_maps = make_in_maps(
        np.asarray(inputs["last_hiddens"]),
        np.asarray(inputs["image_features"]),
        np.asarray(inputs["Wa_w"]),
        np.asarray(inputs["Wa_b"]),
        np.asarray(inputs["Ua_w"]),
        np.asarray(inputs["Ua_b"]),
        np.asarray(inputs["va_w"]),
    )
    nc = _get_nc()
    res = run_bass_kernel_spmd(
        nc, in_maps, core_ids=list(range(NCORES)), trace=trace, **spmd_kwargs
    )
    return _assemble(res.results), res


def kernel(**inputs):
    outputs, _ = run(inputs)
    return outputs


# revision 22
# speedup vs baseline: 1.1504x; 1.0108x over previous
"""Bahdanau-style additive attention (nn_Attentionv2) on 8 Trainium2 NeuronCores.

Reference computation (per batch b):
    b_img[s,a] = img[s,:] @ Wa^T + Wa_b          # [S, A]
    b_hid[t,a] = hid[t,:] @ Ua^T + Ua_b          # [T, A]
    e[t,s,a]   = tanh(b_img[s,a] + b_hid[t,a])
    scores[t,s]= sum_a va[a]*e[t,s,a] (+ va_b)   # va_b dropped: softmax-invariant
    w          = softmax_s(scores)               # mask is all-ones -> no-op
    context    = w @ img                         # [T, F]
Outputs: context [T,B,F], weights [T,B,S,1].

Sharding: data-parallel over B (8 batches -> 8 cores), one batch per core.

Per-core design (T=128, S=512, F=H=512, A=256). The kernel is ScalarE-bound:
16.8M tanh evaluations at 128 lanes / 1.2 GHz ~= 110us is the floor, so the
structure keeps ACT saturated from ~10us on and hides everything else:
  - fp16 throughout the attention pipeline (fp16 keeps ~11 mantissa bits on
    values that all live in [-10, 10]; measured end-to-end rel err ~3e-4).
    fp16/bf16 matmuls are single-pass on the PE (fp32 lowers to LOW/HIGH
    double passes) and get fast weight loads.
  - a dummy activation at t=0 pulls the ~2.7us ACT table load off the
    critical path.
  - e in [a_chunk(128 part), (t, s)] layout: broadcast add b_img + b_hid[t]
    is a DVE tensor_scalar (per-partition scalar), tanh is one big ACT
    instruction per group of t's (in place). Group sizes ramp 2,2,4,8,16...
    so ACT starts early, and ramp down at the end to shorten the tail.
  - scoresT [s(4x128 part), t] via N=1 matmuls: lhsT = e tile [a,128 s],
    rhs = va [a,1], accumulated over the two a-chunks in PSUM.
  - softmax without max-subtraction (|scores| <= sum|va| ~ 8; exp is safe in
    fp32): exp on ACT, row sums via PE ones-matmul (partition reduce),
    normalization on DVE. Done in two t-halves so the first half overlaps
    the second half of the main loop.
  - context = wT.T @ img directly (wT is already the needed lhsT layout);
    the weights output needs [t,s] so wT is PE-transposed before DMA out.
"""

import numpy as np

T, S, F, H, A = 128, 512, 512, 512, 256
B = 8
NCORES = 8

# group sizes for the tanh pipeline (sum = T)
GROUPS = [2, 2, 4, 8] + [16] * 6 + [8, 4, 2, 2]
assert sum(GROUPS) == T
GT_MAX = max(GROUPS)

_CACHE = {}


def _build():
    """Build the Bass module (one NeuronCore program, SPMD across 8 cores)."""
    import concourse.bacc as bacc
    import concourse.bass as bass
    import concourse.tile as tile
    from concourse import mybir
    from concourse.masks import make_identity

    f32 = mybir.dt.float32
    f16 = mybir.dt.float16

    nc = bacc.Bacc("TRN2")

    # fp16 inputs (host-converted): prologue + context operands
    imgT_d = nc.declare_dram_parameter("imgT", [F, S], f16, isOutput=False)
    img_d = nc.declare_dram_parameter("img", [S, F], f16, isOutput=False)
    hidT_d = nc.declare_dram_parameter("hidT", [H, T], f16, isOutput=False)
    WaT_d = nc.declare_dram_parameter("WaT", [F, A], f16, isOutput=False)
    UaT_d = nc.declare_dram_parameter("UaT", [H, A], f16, isOutput=False)
    va_d = nc.declare_dram_parameter("va2", [128, 2], f16, isOutput=False)
    bias_d = nc.declare_dram_parameter("bias2", [128, 2], f32, isOutput=False)
    ctx_d = nc.declare_dram_parameter("ctx", [T, F], f32, isOutput=True)
    wts_d = nc.declare_dram_parameter("wts", [T, S], f32, isOutput=True)

    with tile.TileContext(nc) as tc:
        with (
            tc.tile_pool(name="const", bufs=1) as const,
            tc.tile_pool(name="work", bufs=4) as work,
            tc.tile_pool(name="misc", bufs=1) as misc,
        ):
            # dummy activation: pull the ACT table load to t=0
            dummy = const.tile([128, 1], f32)
            nc.vector.memset(dummy[:], 0.0)
            nc.scalar.activation(dummy[:], dummy[:], mybir.ActivationFunctionType.Tanh)

            # DMA order = criticality: the b_img path (imgT, WaT) gates the
            # first tanh; img/va go on the scalar HWDGE queue (not urgent)
            imgT_sb = const.tile([128, 4, S], f16)
            nc.sync.dma_start(imgT_sb[:], imgT_d.rearrange("(c p) s -> p c s", p=128))
            WaT_sb = const.tile([128, 4, A], f16)
            nc.sync.dma_start(WaT_sb[:], WaT_d.rearrange("(c p) a -> p c a", p=128))
            hidT_sb = const.tile([128, 4, T], f16)
            nc.sync.dma_start(hidT_sb[:], hidT_d.rearrange("(c p) t -> p c t", p=128))
            UaT_sb = const.tile([128, 4, A], f16)
            nc.sync.dma_start(UaT_sb[:], UaT_d.rearrange("(c p) a -> p c a", p=128))
            bias_sb = const.tile([128, 2], f32)
            nc.sync.dma_start(bias_sb[:], bias_d[:, :])
            va_sb = const.tile([128, 2], f16)
            nc.sync.dma_start(va_sb[:], va_d[:, :])
            img_sb = const.tile([128, 4, F], f16)
            nc.sync.dma_start(img_sb[:], img_d.rearrange("(c p) f -> p c f", p=128))
            ident = const.tile([128, 128], f16)
            make_identity(nc, ident[:])
            ones_sb = const.tile([128, 128], f16)
            nc.vector.memset(ones_sb[:], 1.0)

            bimg_sb = misc.tile([128, 2, S], f16)  # [a_chunk part, (chunk, s)]
            bhid_sb = misc.tile([128, 2, T], f32)  # [a_chunk part, (chunk, t)]

            # prologue: b_img = Wa@img^T, b_hid = Ua@hid^T (+ combined bias)
            with tc.tile_pool(name="psum_pre", bufs=2, space="PSUM") as pp:
                for c in range(2):
                    t_img = pp.tile([128, S], f32, tag="pimg")
                    for k in range(4):
                        nc.tensor.matmul(
                            t_img[:],
                            WaT_sb[:, k, c * 128 : (c + 1) * 128],
                            imgT_sb[:, k, :],
                            start=(k == 0),
                            stop=(k == 3),
                        )
                    nc.vector.tensor_copy(bimg_sb[:, c, :], t_img[:])
                    t_hid = pp.tile([128, T], f32, tag="phid")
                    for k in range(4):
                        nc.tensor.matmul(
                            t_hid[:],
                            UaT_sb[:, k, c * 128 : (c + 1) * 128],
                            hidT_sb[:, k, :],
                            start=(k == 0),
                            stop=(k == 3),
                        )
                    # bias (Wa_b + Ua_b) folded into b_hid in the PSUM->SBUF copy
                    nc.vector.tensor_scalar_add(
                        bhid_sb[:, c, :], t_hid[:], bias_sb[:, c : c + 1]
                    )

            pmain_cm = tc.tile_pool(name="psum_main", bufs=1, space="PSUM")
            pmain = pmain_cm.__enter__()
            # scoresT[s, t] accumulates here: [s%128 part, (s_blk, t)] = 1 bank
            scT = pmain.tile([128, 4, T], f32)

            # epilogue state (written per t-half)
            expT_sb = misc.tile([128, 4, T], f16)

            def epilogue_half(h):
                tsl = slice(h * 64, h * 64 + 64)
                # softmax (no max subtraction): exp reads scoresT from PSUM
                nc.scalar.activation(
                    expT_sb[:, :, tsl], scT[:, :, tsl],
                    mybir.ActivationFunctionType.Exp,
                )
                # unnormalized context runs concurrently with the sums path;
                # normalization is fused into the PSUM->SBUF copies below.
                ctx_ps = pmain.tile([64, F], f32, tag="ctx")
                for c in range(4):
                    nc.tensor.matmul(
                        ctx_ps[:],
                        expT_sb[:, c, tsl],
                        img_sb[:, c, :],
                        start=(c == 0),
                        stop=(c == 3),
                    )
                # row sums: ones-matmul partition reduce, then PE-transpose the
                # (replicated) sums so 1/sums lands on partition t
                sums_ps = pmain.tile([128, 64], f32, tag="sums")
                for c in range(4):
                    nc.tensor.matmul(
                        sums_ps[:],
                        ones_sb[:],
                        expT_sb[:, c, tsl],
                        start=(c == 0),
                        stop=(c == 3),
                    )
                sums16_sb = misc.tile([128, 64], f16, tag="sums16")
                nc.vector.tensor_copy(sums16_sb[:], sums_ps[:])
                sumsT_ps = pmain.tile([64, 128], f16, tag="sumsT")
                nc.tensor.transpose(sumsT_ps[:], sums16_sb[:], ident[:])
                recipT = misc.tile([64, 1], f32, tag="recipT")
                nc.vector.reciprocal(recipT[:], sumsT_ps[:, 0:1])

                ctx_sb = misc.tile([64, F], f32, tag=f"ctxsb{h}")
                nc.vector.tensor_scalar_mul(ctx_sb[:], ctx_ps[:], recipT[:])
                nc.sync.dma_start(ctx_d[tsl, :], ctx_sb[:])
                # weights output needs [t, s]: PE-transpose each expT block,
                # scale by 1/sums during the PSUM->SBUF copy
                wtp_ps = pmain.tile([64, 4, 128], f16, tag="wtp")
                w_sb = misc.tile([64, 4, 128], f32, tag=f"wsb{h}")
                for c in range(4):
                    nc.tensor.transpose(
                        wtp_ps[:, c, :], expT_sb[:, c, tsl], ident[:]
                    )
                    nc.vector.tensor_scalar_mul(
                        w_sb[:, c, :], wtp_ps[:, c, :], recipT[:]
                    )
                nc.sync.dma_start(
                    wts_d[tsl, :], w_sb[:].rearrange("p c s -> p (c s)")
                )

            # main loop over t-groups
            t0 = 0
            for gi, gt in enumerate(GROUPS):
                arg = work.tile([128, GT_MAX, 2, S], f16, tag="arg")
                for i in range(gt):
                    t = t0 + i
                    for c in range(2):
                        # arg[a, s] = b_img[a, s] + b_hid[a, t]
                        nc.vector.tensor_scalar_add(
                            arg[:, i, c, :], bimg_sb[:, c, :], bhid_sb[:, c, t : t + 1]
                        )
                # one tanh instruction per group (in place)
                nc.scalar.activation(
                    arg[:, 0:gt, :, :], arg[:, 0:gt, :, :],
                    mybir.ActivationFunctionType.Tanh,
                )
                # va-reduce: scoresT[:, sb, t] = sum_a va[a] * e[a, t, sb]
                for i in range(gt):
                    t = t0 + i
                    for sb in range(4):
                        for c in range(2):
                            nc.tensor.matmul(
                                scT[:, sb, t : t + 1],
                                arg[:, i, c, sb * 128 : (sb + 1) * 128],
                                va_sb[:, c : c + 1],
                                start=(c == 0),
                                stop=(c == 1),
                            )
                t0 += gt
                if t0 == 64:
                    epilogue_half(0)
            epilogue_half(1)
            pmain_cm.__exit__(None, None, None)

    nc.compile()
    return nc


def _get_nc():
    if "nc" not in _CACHE:
        _CACHE["nc"] = _build()
    return _CACHE["nc"]


def make_in_maps(last_hiddens, image_features, Wa_w, Wa_b, Ua_w, Ua_b, va_w):
    """Host-side sharding + pre-transposes. One map per core (= per batch)."""
    f = np.float32
    h = np.float16
    WaT = np.ascontiguousarray(Wa_w.T).astype(h)  # [F, A]
    UaT = np.ascontiguousarray(Ua_w.T).astype(h)  # [H, A]
    va2 = np.ascontiguousarray(va_w[0].reshape(2, 128).T).astype(h)  # [128, 2]
    bias2 = np.ascontiguousarray(
        (Wa_b + Ua_b).reshape(2, 128).T, dtype=f
    )  # [128, 2]
    in_maps = []
    for b in range(NCORES):
        img_b = np.ascontiguousarray(image_features[:, b, :])  # [S, F]
        in_maps.append(
            {
                "imgT": np.ascontiguousarray(img_b.T).astype(h),  # [F, S]
                "img": img_b.astype(h),
                "hidT": np.ascontiguousarray(last_hiddens[:, b, :].T).astype(h),
                "WaT": WaT,
                "UaT": UaT,
                "va2": va2,
                "bias2": bias2,
            }
        )
    return in_maps


def _assemble(results):
    context = np.empty((T, B, F), np.float32)
    weights = np.empty((T, B, S, 1), np.float32)
    for b in range(B):
        context[:, b, :] = results[b]["ctx"]
        weights[:, b, :, 0] = results[b]["wts"]
    return context, weights


def run(inputs, trace=False, **spmd_kwargs):
    """Compile (cached) + run on cores 0-7. Returns (outputs, BassKernelResults)."""
    from concourse.bass_utils import run_bass_kernel_spmd

    in_maps = make_in_maps(
        np.asarray(inputs["last_hiddens"]),
        np.asarray(inputs["image_features"]),
        np.asarray(inputs["Wa_w"]),
        np.asarray(inputs["Wa_b"]),
        np.asarray(inputs["Ua_w"]),
        np.asarray(inputs["Ua_b"]),
        np.asarray(inputs["va_w"]),
    )
    nc = _get_nc()
    res = run_bass_kernel_spmd(
        nc, in_maps, core_ids=list(range(NCORES)), trace=trace, **spmd_kwargs
    )
    return _assemble(res.results), res


def kernel(**inputs):
    outputs, _ = run(inputs)
    return outputs


# revision 24
# speedup vs baseline: 1.1821x; 1.0275x over previous
"""Bahdanau-style additive attention (nn_Attentionv2) on 8 Trainium2 NeuronCores.

Reference computation (per batch b):
    b_img[s,a] = img[s,:] @ Wa^T + Wa_b          # [S, A]
    b_hid[t,a] = hid[t,:] @ Ua^T + Ua_b          # [T, A]
    e[t,s,a]   = tanh(b_img[s,a] + b_hid[t,a])
    scores[t,s]= sum_a va[a]*e[t,s,a] (+ va_b)   # va_b dropped: softmax-invariant
    w          = softmax_s(scores)               # mask is all-ones -> no-op
    context    = w @ img                         # [T, F]
Outputs: context [T,B,F], weights [T,B,S,1].

Sharding: data-parallel over B (8 batches -> 8 cores), one batch per core.

Per-core design (T=128, S=512, F=H=512, A=256). The kernel is ScalarE-bound:
16.8M tanh evaluations at 128 lanes / 1.2 GHz ~= 110us is the floor, so the
structure keeps ACT saturated from ~10us on and hides everything else:
  - fp16 throughout the attention pipeline (fp16 keeps ~11 mantissa bits on
    values that all live in [-10, 10]; measured end-to-end rel err ~3e-4).
    fp16/bf16 matmuls are single-pass on the PE (fp32 lowers to LOW/HIGH
    double passes) and get fast weight loads.
  - a dummy activation at t=0 pulls the ~2.7us ACT table load off the
    critical path.
  - e in [a_chunk(128 part), (t, s)] layout: broadcast add b_img + b_hid[t]
    is a DVE tensor_scalar (per-partition scalar), tanh is one big ACT
    instruction per group of t's (in place). Group sizes ramp 2,2,4,8,16...
    so ACT starts early, and ramp down at the end to shorten the tail.
  - scoresT [s(4x128 part), t] via N=1 matmuls: lhsT = e tile [a,128 s],
    rhs = va [a,1], accumulated over the two a-chunks in PSUM.
  - softmax without max-subtraction (|scores| <= sum|va| ~ 8; exp is safe in
    fp32): exp on ACT, row sums via PE ones-matmul (partition reduce),
    normalization on DVE. Done in two t-halves so the first half overlaps
    the second half of the main loop.
  - context = wT.T @ img directly (wT is already the needed lhsT layout);
    the weights output needs [t,s] so wT is PE-transposed before DMA out.
"""

import numpy as np

T, S, F, H, A = 128, 512, 512, 512, 256
B = 8
NCORES = 8

# group sizes for the tanh pipeline (sum = T)
GROUPS = [2, 2, 4, 8] + [16] * 6 + [8, 4, 2, 2]
assert sum(GROUPS) == T
GT_MAX = max(GROUPS)

_CACHE = {}


def _build():
    """Build the Bass module (one NeuronCore program, SPMD across 8 cores)."""
    import concourse.bacc as bacc
    import concourse.bass as bass
    import concourse.tile as tile
    from concourse import mybir
    from concourse.masks import make_identity

    f32 = mybir.dt.float32
    f16 = mybir.dt.float16

    nc = bacc.Bacc("TRN2")

    # fp16 inputs (host-converted): prologue + context operands
    imgT_d = nc.declare_dram_parameter("imgT", [F, S], f16, isOutput=False)
    img_d = nc.declare_dram_parameter("img", [S, F], f16, isOutput=False)
    hidT_d = nc.declare_dram_parameter("hidT", [H, T], f16, isOutput=False)
    WaT_d = nc.declare_dram_parameter("WaT", [F, A], f16, isOutput=False)
    UaT_d = nc.declare_dram_parameter("UaT", [H, A], f16, isOutput=False)
    va_d = nc.declare_dram_parameter("va2", [128, 2], f16, isOutput=False)
    bias_d = nc.declare_dram_parameter("bias2", [128, 2], f32, isOutput=False)
    ctx_d = nc.declare_dram_parameter("ctx", [T, F], f32, isOutput=True)
    wts_d = nc.declare_dram_parameter("wts", [T, S], f32, isOutput=True)

    with tile.TileContext(nc) as tc:
        with (
            tc.tile_pool(name="const", bufs=1) as const,
            tc.tile_pool(name="work", bufs=4) as work,
            tc.tile_pool(name="misc", bufs=1) as misc,
        ):
            # dummy activation: pull the ACT table load to t=0
            dummy = const.tile([128, 1], f32)
            nc.vector.memset(dummy[:], 0.0)
            nc.scalar.activation(dummy[:], dummy[:], mybir.ActivationFunctionType.Tanh)

            # DMA order = criticality: the b_img path (imgT, WaT) gates the
            # first tanh; img/va go on the scalar HWDGE queue (not urgent)
            imgT_sb = const.tile([128, 4, S], f16)
            nc.sync.dma_start(imgT_sb[:], imgT_d.rearrange("(c p) s -> p c s", p=128))
            WaT_sb = const.tile([128, 4, A], f16)
            nc.sync.dma_start(WaT_sb[:], WaT_d.rearrange("(c p) a -> p c a", p=128))
            hidT_sb = const.tile([128, 4, T], f16)
            nc.sync.dma_start(hidT_sb[:], hidT_d.rearrange("(c p) t -> p c t", p=128))
            UaT_sb = const.tile([128, 4, A], f16)
            nc.sync.dma_start(UaT_sb[:], UaT_d.rearrange("(c p) a -> p c a", p=128))
            bias_sb = const.tile([128, 2], f32)
            nc.sync.dma_start(bias_sb[:], bias_d[:, :])
            va_sb = const.tile([128, 2], f16)
            nc.sync.dma_start(va_sb[:], va_d[:, :])
            img_sb = const.tile([128, 4, F], f16)
            nc.sync.dma_start(img_sb[:], img_d.rearrange("(c p) f -> p c f", p=128))
            ident = const.tile([128, 128], f16)
            make_identity(nc, ident[:])
            ones_sb = const.tile([128, 128], f16)
            nc.vector.memset(ones_sb[:], 1.0)

            bimg_sb = misc.tile([128, 2, S], f16)  # [a_chunk part, (chunk, s)]
            bhid_sb = misc.tile([128, 2, T], f32)  # [a_chunk part, (chunk, t)]

            # prologue: b_img = Wa@img^T, b_hid = Ua@hid^T (+ combined bias)
            with tc.tile_pool(name="psum_pre", bufs=2, space="PSUM") as pp:
                for c in range(2):
                    t_img = pp.tile([128, S], f32, tag="pimg")
                    for k in range(4):
                        nc.tensor.matmul(
                            t_img[:],
                            WaT_sb[:, k, c * 128 : (c + 1) * 128],
                            imgT_sb[:, k, :],
                            start=(k == 0),
                            stop=(k == 3),
                        )
                    nc.vector.tensor_copy(bimg_sb[:, c, :], t_img[:])
                    t_hid = pp.tile([128, T], f32, tag="phid")
                    for k in range(4):
                        nc.tensor.matmul(
                            t_hid[:],
                            UaT_sb[:, k, c * 128 : (c + 1) * 128],
                            hidT_sb[:, k, :],
                            start=(k == 0),
                            stop=(k == 3),
                        )
                    # bias (Wa_b + Ua_b) folded into b_hid in the PSUM->SBUF copy
                    nc.vector.tensor_scalar_add(
                        bhid_sb[:, c, :], t_hid[:], bias_sb[:, c : c + 1]
                    )

            pmain_cm = tc.tile_pool(name="psum_main", bufs=1, space="PSUM")
            pmain = pmain_cm.__enter__()
            # scoresT[s, t] accumulates here: [s%128 part, (s_blk, t)] = 1 bank
            scT = pmain.tile([128, 4, T], f32)

            # epilogue state (written per t-half)
            expT_sb = misc.tile([128, 4, T], f16)

            def epilogue(lo, hi):
                tsl = slice(lo, hi)
                W = hi - lo
                # softmax (no max subtraction): exp reads scoresT from PSUM
                nc.scalar.activation(
                    expT_sb[:, :, tsl], scT[:, :, tsl],
                    mybir.ActivationFunctionType.Exp,
                )
                # row sums first (they gate the reciprocal chain): ones-matmul
                # partition reduce, then PE-transpose the (replicated) sums so
                # 1/sums lands on partition t
                sums_ps = pmain.tile([128, W], f32, tag="sums")
                for c in range(4):
                    nc.tensor.matmul(
                        sums_ps[:],
                        ones_sb[:],
                        expT_sb[:, c, tsl],
                        start=(c == 0),
                        stop=(c == 3),
                    )
                sums16_sb = misc.tile([128, W], f16, tag="sums16")
                nc.vector.tensor_copy(sums16_sb[:], sums_ps[:])
                sumsT_ps = pmain.tile([W, 128], f16, tag="sumsT")
                nc.tensor.transpose(sumsT_ps[:], sums16_sb[:], ident[:])
                recipT = misc.tile([W, 1], f32, tag="recipT")
                nc.vector.reciprocal(recipT[:], sumsT_ps[:, 0:1])
                # unnormalized context runs concurrently with the sums path;
                # normalization is fused into the PSUM->SBUF copies below.
                ctx_ps = pmain.tile([W, F], f32, tag="ctx")
                for c in range(4):
                    nc.tensor.matmul(
                        ctx_ps[:],
                        expT_sb[:, c, tsl],
                        img_sb[:, c, :],
                        start=(c == 0),
                        stop=(c == 3),
                    )
                ctx_sb = misc.tile([W, F], f32, tag=f"ctxsb{lo}")
                nc.vector.tensor_scalar_mul(ctx_sb[:], ctx_ps[:], recipT[:])
                nc.sync.dma_start(ctx_d[tsl, :], ctx_sb[:])
                # weights output needs [t, s]: PE-transpose each expT block,
                # scale by 1/sums during the PSUM->SBUF copy (per-chunk PSUM
                # banks so transposes and scales pipeline)
                w_sb = misc.tile([W, 4, 128], f32, tag=f"wsb{lo}")
                for c in range(4):
                    wtp_ps = pmain.tile([W, 128], f16, tag=f"wtp{c}")
                    nc.tensor.transpose(
                        wtp_ps[:], expT_sb[:, c, tsl], ident[:]
                    )
                    nc.vector.tensor_scalar_mul(
                        w_sb[:, c, :], wtp_ps[:], recipT[:]
                    )
                nc.sync.dma_start(
                    wts_d[tsl, :], w_sb[:].rearrange("p c s -> p (c s)")
                )

            # main loop over t-groups
            t0 = 0
            for gi, gt in enumerate(GROUPS):
                arg = work.tile([128, GT_MAX, 2, S], f16, tag="arg")
                for i in range(gt):
                    t = t0 + i
                    for c in range(2):
                        # arg[a, s] = b_img[a, s] + b_hid[a, t]
                        nc.vector.tensor_scalar_add(
                            arg[:, i, c, :], bimg_sb[:, c, :], bhid_sb[:, c, t : t + 1]
                        )
                # one tanh instruction per group (in place)
                nc.scalar.activation(
                    arg[:, 0:gt, :, :], arg[:, 0:gt, :, :],
                    mybir.ActivationFunctionType.Tanh,
                )
                # va-reduce: scoresT[:, sb, t] = sum_a va[a] * e[a, t, sb]
                for i in range(gt):
                    t = t0 + i
                    for sb in range(4):
                        for c in range(2):
                            nc.tensor.matmul(
                                scT[:, sb, t : t + 1],
                                arg[:, i, c, sb * 128 : (sb + 1) * 128],
                                va_sb[:, c : c + 1],
                                start=(c == 0),
                                stop=(c == 1),
                            )
                t0 += gt
                if t0 == 64:
                    epilogue(0, 64)
                elif t0 == 112:
                    epilogue(64, 112)
            epilogue(112, 128)
            pmain_cm.__exit__(None, None, None)

    nc.compile()
    return nc


def _get_nc():
    if "nc" not in _CACHE:
        _CACHE["nc"] = _build()
    return _CACHE["nc"]


def make_in_maps(last_hiddens, image_features, Wa_w, Wa_b, Ua_w, Ua_b, va_w):
    """Host-side sharding + pre-transposes. One map per core (= per batch)."""
    f = np.float32
    h = np.float16
    WaT = np.ascontiguousarray(Wa_w.T).astype(h)  # [F, A]
    UaT = np.ascontiguousarray(Ua_w.T).astype(h)  # [H, A]
    va2 = np.ascontiguousarray(va_w[0].reshape(2, 128).T).astype(h)  # [128, 2]
    bias2 = np.ascontiguousarray(
        (Wa_b + Ua_b).reshape(2, 128).T, dtype=f
    )  # [128, 2]
    in_maps = []
    for b in range(NCORES):
        img_b = np.ascontiguousarray(image_features[:, b, :])  # [S, F]
        in_maps.append(
            {
                "imgT": np.ascontiguousarray(img_b.T).astype(h),  # [F, S]
                "img": img_b.astype(h),
                "hidT": np.ascontiguousarray(last_hiddens[:, b, :].T).astype(h),
                "WaT": WaT,
                "UaT": UaT,
                "va2": va2,
                "bias2": bias2,
            }
        )
    return in_maps


def _assemble(results):
    context = np.empty((T, B, F), np.float32)
    weights = np.empty((T, B, S, 1), np.float32)
    for b in range(B):
        context[:, b, :] = results[b]["ctx"]
        weights[:, b, :, 0] = results[b]["wts"]
    return context, weights


def run(inputs, trace=False, **spmd_kwargs):
    """Compile (cached) + run on cores 0-7. Returns (outputs, BassKernelResults)."""
    from concourse.bass_utils import run_bass_kernel_spmd

    in_maps = make_in_maps(
        np.asarray(inputs["last_hiddens"]),
        np.asarray(inputs["image_features"]),
        np.asarray(inputs["Wa_w"]),
        np.asarray(inputs["Wa_b"]),
        np.asarray(inputs["Ua_w"]),
        np.asarray(inputs["Ua_b"]),
        np.asarray(inputs["va_w"]),
    )
    nc = _get_nc()
    res = run_bass_kernel_spmd(
        nc, in_maps, core_ids=list(range(NCORES)), trace=trace, **spmd_kwargs
    )
    return _assemble(res.results), res


def kernel(**inputs):
    outputs, _ = run(inputs)
    return outputs


# revision 28
# speedup vs baseline: 1.1888x; 1.0056x over previous
"""Bahdanau-style additive attention (nn_Attentionv2) on 8 Trainium2 NeuronCores.

Reference computation (per batch b):
    b_img[s,a] = img[s,:] @ Wa^T + Wa_b          # [S, A]
    b_hid[t,a] = hid[t,:] @ Ua^T + Ua_b          # [T, A]
    e[t,s,a]   = tanh(b_img[s,a] + b_hid[t,a])
    scores[t,s]= sum_a va[a]*e[t,s,a] (+ va_b)   # va_b dropped: softmax-invariant
    w          = softmax_s(scores)               # mask is all-ones -> no-op
    context    = w @ img                         # [T, F]
Outputs: context [T,B,F], weights [T,B,S,1].

Sharding: data-parallel over B (8 batches -> 8 cores), one batch per core.

Per-core design (T=128, S=512, F=H=512, A=256). The kernel is ScalarE-bound:
16.8M tanh evaluations at 128 lanes / 1.2 GHz ~= 110us is the floor, so the
structure keeps ACT saturated from ~10us on and hides everything else:
  - fp16 throughout the attention pipeline (fp16 keeps ~11 mantissa bits on
    values that all live in [-10, 10]; measured end-to-end rel err ~3e-4).
    fp16/bf16 matmuls are single-pass on the PE (fp32 lowers to LOW/HIGH
    double passes) and get fast weight loads.
  - a dummy activation at t=0 pulls the ~2.7us ACT table load off the
    critical path.
  - e in [a_chunk(128 part), (t, s)] layout: broadcast add b_img + b_hid[t]
    is a DVE tensor_scalar (per-partition scalar), tanh is one big ACT
    instruction per group of t's (in place). Group sizes ramp 2,2,4,8,16...
    so ACT starts early, and ramp down at the end to shorten the tail.
  - scoresT [s(4x128 part), t] via N=1 matmuls: lhsT = e tile [a,128 s],
    rhs = va [a,1], accumulated over the two a-chunks in PSUM.
  - softmax without max-subtraction (|scores| <= sum|va| ~ 8; exp is safe in
    fp32): exp on ACT, row sums via PE ones-matmul (partition reduce),
    normalization on DVE. Done in two t-halves so the first half overlaps
    the second half of the main loop.
  - context = wT.T @ img directly (wT is already the needed lhsT layout);
    the weights output needs [t,s] so wT is PE-transposed before DMA out.
"""

import numpy as np

T, S, F, H, A = 128, 512, 512, 512, 256
B = 8
NCORES = 8

# group sizes for the tanh pipeline (sum = T); epilogue fires at cum 64/112
GROUPS = [2, 2, 4, 8, 12, 16, 16, 4, 16, 16, 16, 8, 4, 2, 2]
assert sum(GROUPS) == T
GT_MAX = max(GROUPS)
CUM_SPLITS = (64, 112)

_CACHE = {}


def _build():
    """Build the Bass module (one NeuronCore program, SPMD across 8 cores)."""
    import concourse.bacc as bacc
    import concourse.bass as bass
    import concourse.tile as tile
    from concourse import mybir
    from concourse.masks import make_identity

    f32 = mybir.dt.float32
    f16 = mybir.dt.float16

    nc = bacc.Bacc("TRN2")

    # fp16 inputs (host-converted): prologue + context operands
    imgT_d = nc.declare_dram_parameter("imgT", [F, S], f16, isOutput=False)
    img_d = nc.declare_dram_parameter("img", [S, F], f16, isOutput=False)
    hidT_d = nc.declare_dram_parameter("hidT", [H, T], f16, isOutput=False)
    WaT_d = nc.declare_dram_parameter("WaT", [F, A], f16, isOutput=False)
    UaT_d = nc.declare_dram_parameter("UaT", [H, A], f16, isOutput=False)
    va_d = nc.declare_dram_parameter("va2", [128, 2], f16, isOutput=False)
    bias_d = nc.declare_dram_parameter("bias2", [128, 2], f32, isOutput=False)
    ctx_d = nc.declare_dram_parameter("ctx", [T, F], f32, isOutput=True)
    wts_d = nc.declare_dram_parameter("wts", [T, S], f32, isOutput=True)

    with tile.TileContext(nc) as tc:
        with (
            tc.tile_pool(name="const", bufs=1) as const,
            tc.tile_pool(name="work", bufs=4) as work,
            tc.tile_pool(name="misc", bufs=1) as misc,
        ):
            # dummy activation: pull the ACT table load to t=0
            dummy = const.tile([128, 1], f32)
            nc.vector.memset(dummy[:], 0.0)
            nc.scalar.activation(dummy[:], dummy[:], mybir.ActivationFunctionType.Tanh)

            # DMA order = criticality: the b_img path (imgT, WaT) gates the
            # first tanh; img/va go on the scalar HWDGE queue (not urgent)
            imgT_sb = const.tile([128, 4, S], f16)
            nc.sync.dma_start(imgT_sb[:], imgT_d.rearrange("(c p) s -> p c s", p=128))
            WaT_sb = const.tile([128, 4, A], f16)
            nc.sync.dma_start(WaT_sb[:], WaT_d.rearrange("(c p) a -> p c a", p=128))
            hidT_sb = const.tile([128, 4, T], f16)
            nc.sync.dma_start(hidT_sb[:], hidT_d.rearrange("(c p) t -> p c t", p=128))
            UaT_sb = const.tile([128, 4, A], f16)
            nc.sync.dma_start(UaT_sb[:], UaT_d.rearrange("(c p) a -> p c a", p=128))
            bias_sb = const.tile([128, 2], f32)
            nc.sync.dma_start(bias_sb[:], bias_d[:, :])
            va_sb = const.tile([128, 2], f16)
            nc.sync.dma_start(va_sb[:], va_d[:, :])
            img_sb = const.tile([128, 4, F], f16)
            nc.sync.dma_start(img_sb[:], img_d.rearrange("(c p) f -> p c f", p=128))
            ident = const.tile([128, 128], f16)
            make_identity(nc, ident[:])
            ones_sb = const.tile([128, 128], f16)
            nc.vector.memset(ones_sb[:], 1.0)

            bimg_sb = misc.tile([128, 2, S], f16)  # [a_chunk part, (chunk, s)]
            bhid_sb = misc.tile([128, 2, T], f32)  # [a_chunk part, (chunk, t)]

            # prologue: b_img = Wa@img^T, b_hid = Ua@hid^T (+ combined bias)
            with tc.tile_pool(name="psum_pre", bufs=2, space="PSUM") as pp:
                for c in range(2):
                    t_img = pp.tile([128, S], f32, tag="pimg")
                    for k in range(4):
                        nc.tensor.matmul(
                            t_img[:],
                            WaT_sb[:, k, c * 128 : (c + 1) * 128],
                            imgT_sb[:, k, :],
                            start=(k == 0),
                            stop=(k == 3),
                        )
                    nc.vector.tensor_copy(bimg_sb[:, c, :], t_img[:])
                    t_hid = pp.tile([128, T], f32, tag="phid")
                    for k in range(4):
                        nc.tensor.matmul(
                            t_hid[:],
                            UaT_sb[:, k, c * 128 : (c + 1) * 128],
                            hidT_sb[:, k, :],
                            start=(k == 0),
                            stop=(k == 3),
                        )
                    # bias (Wa_b + Ua_b) folded into b_hid in the PSUM->SBUF copy
                    nc.vector.tensor_scalar_add(
                        bhid_sb[:, c, :], t_hid[:], bias_sb[:, c : c + 1]
                    )

            pmain_cm = tc.tile_pool(name="psum_main", bufs=1, space="PSUM")
            pmain = pmain_cm.__enter__()
            # scoresT[s, t] accumulates here: [s%128 part, (s_blk, t)] = 1 bank
            scT = pmain.tile([128, 4, T], f32)

            # epilogue state (written per t-half)
            expT_sb = misc.tile([128, 4, T], f16)

            def epilogue(lo, hi):
                tsl = slice(lo, hi)
                W = hi - lo
                # softmax (no max subtraction): exp reads scoresT from PSUM
                nc.scalar.activation(
                    expT_sb[:, :, tsl], scT[:, :, tsl],
                    mybir.ActivationFunctionType.Exp,
                )
                # row sums first (they gate the reciprocal chain): ones-matmul
                # partition reduce, then PE-transpose the (replicated) sums so
                # 1/sums lands on partition t
                sums_ps = pmain.tile([128, W], f32, tag="sums")
                for c in range(4):
                    nc.tensor.matmul(
                        sums_ps[:],
                        ones_sb[:],
                        expT_sb[:, c, tsl],
                        start=(c == 0),
                        stop=(c == 3),
                    )
                sums16_sb = misc.tile([128, W], f16, tag="sums16")
                nc.vector.tensor_copy(sums16_sb[:], sums_ps[:])
                sumsT_ps = pmain.tile([W, 128], f16, tag="sumsT")
                nc.tensor.transpose(sumsT_ps[:], sums16_sb[:], ident[:])
                recipT = misc.tile([W, 1], f32, tag="recipT")
                nc.vector.reciprocal(recipT[:], sumsT_ps[:, 0:1])
                # unnormalized context runs concurrently with the sums path;
                # normalization is fused into the PSUM->SBUF copies below.
                ctx_ps = pmain.tile([W, F], f32, tag="ctx")
                for c in range(4):
                    nc.tensor.matmul(
                        ctx_ps[:],
                        expT_sb[:, c, tsl],
                        img_sb[:, c, :],
                        start=(c == 0),
                        stop=(c == 3),
                    )
                ctx_sb = misc.tile([W, F], f32, tag=f"ctxsb{lo}")
                nc.vector.tensor_scalar_mul(ctx_sb[:], ctx_ps[:], recipT[:])
                nc.sync.dma_start(ctx_d[tsl, :], ctx_sb[:])
                # weights output needs [t, s]: PE-transpose each expT block,
                # scale by 1/sums during the PSUM->SBUF copy (per-chunk PSUM
                # banks so transposes and scales pipeline)
                w_sb = misc.tile([W, 4, 128], f32, tag=f"wsb{lo}")
                for c in range(4):
                    wtp_ps = pmain.tile([W, 128], f16, tag=f"wtp{c}")
                    nc.tensor.transpose(
                        wtp_ps[:], expT_sb[:, c, tsl], ident[:]
                    )
                    nc.vector.tensor_scalar_mul(
                        w_sb[:, c, :], wtp_ps[:], recipT[:]
                    )
                nc.sync.dma_start(
                    wts_d[tsl, :], w_sb[:].rearrange("p c s -> p (c s)")
                )

            # main loop over t-groups
            t0 = 0
            for gi, gt in enumerate(GROUPS):
                arg = work.tile([128, GT_MAX, 2, S], f16, tag="arg")
                for c in range(2):
                    for i in range(gt):
                        t = t0 + i
                        # arg[a, s] = b_img[a, s] + b_hid[a, t]
                        nc.vector.tensor_scalar_add(
                            arg[:, i, c, :], bimg_sb[:, c, :], bhid_sb[:, c, t : t + 1]
                        )
                # one tanh instruction per group (in place); during the ramp,
                # split per a-chunk so ACT starts on chunk 0 before chunk 1's
                # adds (and b_img chunk 1) are ready
                if t0 < 16:
                    for c in range(2):
                        nc.scalar.activation(
                            arg[:, 0:gt, c, :], arg[:, 0:gt, c, :],
                            mybir.ActivationFunctionType.Tanh,
                        )
                else:
                    nc.scalar.activation(
                        arg[:, 0:gt, :, :], arg[:, 0:gt, :, :],
                        mybir.ActivationFunctionType.Tanh,
                    )
                # va-reduce: scoresT[:, sb, t] = sum_a va[a] * e[a, t, sb]
                for i in range(gt):
                    t = t0 + i
                    for sb in range(4):
                        for c in range(2):
                            nc.tensor.matmul(
                                scT[:, sb, t : t + 1],
                                arg[:, i, c, sb * 128 : (sb + 1) * 128],
                                va_sb[:, c : c + 1],
                                start=(c == 0),
                                stop=(c == 1),
                            )
                t0 += gt
                if t0 == CUM_SPLITS[0]:
                    epilogue(0, CUM_SPLITS[0])
                elif t0 == CUM_SPLITS[1]:
                    epilogue(CUM_SPLITS[0], CUM_SPLITS[1])
            epilogue(CUM_SPLITS[1], T)
            pmain_cm.__exit__(None, None, None)

    nc.compile()
    return nc


def _get_nc():
    if "nc" not in _CACHE:
        _CACHE["nc"] = _build()
    return _CACHE["nc"]


def make_in_maps(last_hiddens, image_features, Wa_w, Wa_b, Ua_w, Ua_b, va_w):
    """Host-side sharding + pre-transposes. One map per core (= per batch)."""
    f = np.float32
    h = np.float16
    WaT = np.ascontiguousarray(Wa_w.T).astype(h)  # [F, A]
    UaT = np.ascontiguousarray(Ua_w.T).astype(h)  # [H, A]
    va2 = np.ascontiguousarray(va_w[0].reshape(2, 128).T).astype(h)  # [128, 2]
    bias2 = np.ascontiguousarray(
        (Wa_b + Ua_b).reshape(2, 128).T, dtype=f
    )  # [128, 2]
    in_maps = []
    for b in range(NCORES):
        img_b = np.ascontiguousarray(image_features[:, b, :])  # [S, F]
        in_maps.append(
            {
                "imgT": np.ascontiguousarray(img_b.T).astype(h),  # [F, S]
                "img": img_b.astype(h),
                "hidT": np.ascontiguousarray(last_hiddens[:, b, :].T).astype(h),
                "WaT": WaT,
                "UaT": UaT,
                "va2": va2,
                "bias2": bias2,
            }
        )
    return in_maps


def _assemble(results):
    context = np.empty((T, B, F), np.float32)
    weights = np.empty((T, B, S, 1), np.float32)
    for b in range(B):
        context[:, b, :] = results[b]["ctx"]
        weights[:, b, :, 0] = results[b]["wts"]
    return context, weights


def run(inputs, trace=False, **spmd_kwargs):
    """Compile (cached) + run on cores 0-7. Returns (outputs, BassKernelResults)."""
    from concourse.bass_utils import run_bass_kernel_spmd

    in_maps = make_in_maps(
        np.asarray(inputs["last_hiddens"]),
        np.asarray(inputs["image_features"]),
        np.asarray(inputs["Wa_w"]),
        np.asarray(inputs["Wa_b"]),
        np.asarray(inputs["Ua_w"]),
        np.asarray(inputs["Ua_b"]),
        np.asarray(inputs["va_w"]),
    )
    nc = _get_nc()
    res = run_bass_kernel_spmd(
        nc, in_maps, core_ids=list(range(NCORES)), trace=trace, **spmd_kwargs
    )
    return _assemble(res.results), res


def kernel(**inputs):
    outputs, _ = run(inputs)
    return outputs
